# revision 3
# baseline (speedup 1.0000x reference)
"""Trainium2 Bass kernel for nn_DecoderRNN (240-step LSTM decoder, B=512, H=2048).

8-way tensor parallel on the hidden/gate dims, batch split into two
software-pipelined halves (256 columns each) - while half A waits on its
collectives, the tensor engine runs half B's matmuls.

Per core: 1024 gate rows (128-row tiles of i/f/g/o x 2 groups), 256 h rows,
256 z rows. Each core's h rows are globally contiguous [j*256,(j+1)*256), so
the rank-major AllGather output is already in natural hidden order.

v3 changes vs the 49ms AllReduce baseline (microbench-driven):
 - Collectives SERIALIZE on the fabric (measured: AG-chain 17.2us + AR-chain
   27.5us interleaved = 40.4us), and an AllReduce costs ~2x a same-payload
   AllGather (RS half does 2 HBM reads per wire byte). The per-step fabric
   budget, 2x(AG+AR) ~ 64us, WAS the bottleneck (71us/step device).
 - So the out AllReduce is replaced by an AllGather of the 8 per-core fc2
   partials (f32, padded to 256 rows for clean APs) + an on-core DVE
   reduction - mathematically identical f32 sum, ~9us/half less fabric.
 - The per-step M1@onehot matmuls are merged into the W_ih tail: stationary
   [77,1024] = [W_ih rows 128:165 ; M1], moving [77,256] = [out(t-1) rows
   128:165 ; onehot]. Matmul cost is set by moving columns, so the K=40
   one-hot term rides for free (-8 MMs/half/step).

Block for step t, half X (hcat = gathered h(t)):
  fc1 -> relu -> fc2 partial -> AllGather out-partials(t-1)   (in flight ...)
  W_hh k-chains -> gates(t) psum (4 banks, 2 m-tiles packed per bank)
  load gathered partials, DVE-sum -> out(t-1), cast to bf16 ob0/obm
  gates(t) += W_ih/[W_ih;M1] @ [out(t-1); onehot]  (tail)
  LSTM cell -> h(t+1) -> DMA -> AllGather (trigger ordered before the outs
  stores on the gpsimd queue)
The gathered h for half X is DMA-loaded during the OTHER half's next block.
Block 0 skips the fc/out part (out(-1) = frame0); an epilogue computes
out(L-1) only.
"""

import sys

if "/opt/trn_rl_repo" not in sys.path:
    sys.path.insert(0, "/opt/trn_rl_repo")

import numpy as np
import ml_dtypes

B = 512
HB = 256        # batch columns per half
OUT = 165
H = 2048
NCLS = 40
NC = 8
BL = B // NC    # output batch columns owned per core
KT = H // 128   # 16 k-tiles over the hidden dim
MR = [128, OUT - 128]

_CACHE = {}


def _build(L):
    import concourse.bacc as bacc
    import concourse.mybir as mybir
    import concourse.tile as tile
    from concourse.bass import ds
    from contextlib import ExitStack

    f32 = mybir.dt.float32
    bf16 = mybir.dt.bfloat16
    AF = mybir.ActivationFunctionType
    RG = [list(range(NC))]

    nc = bacc.Bacc("TRN2", target_bir_lowering=False, debug=False, num_devices=NC)

    whh_d = nc.dram_tensor("whh", [H, 1024], bf16, kind="ExternalInput")
    wih0_d = nc.dram_tensor("wih0", [128, 1024], bf16, kind="ExternalInput")
    wihm_d = nc.dram_tensor("wihm", [77, 1024], bf16, kind="ExternalInput")
    wfc1_d = nc.dram_tensor("wfc1", [H, 256], bf16, kind="ExternalInput")
    wfc2_d = nc.dram_tensor("wfc2", [256, OUT], bf16, kind="ExternalInput")
    onehot_d = nc.dram_tensor("onehot", [NCLS, B], bf16, kind="ExternalInput")
    bgates_d = nc.dram_tensor("bgates", [128, 8], f32, kind="ExternalInput")
    bz_d = nc.dram_tensor("bz", [128, 2], f32, kind="ExternalInput")
    bo_d = nc.dram_tensor("bo", [128, 2], f32, kind="ExternalInput")
    h0_d = nc.dram_tensor("h0", [H, B], bf16, kind="ExternalInput")
    c0_d = nc.dram_tensor("c0", [256, B], f32, kind="ExternalInput")
    out0_d = nc.dram_tensor("out0", [OUT, B], bf16, kind="ExternalInput")
    # cols 0:BL always hold this core's batch slice; cols BL:2*BL are scratch
    # written by the other half's store (keeps the program identical per core).
    outs_d = nc.dram_tensor("outs", [L, OUT, 2 * BL], f32, kind="ExternalOutput")

    with tile.TileContext(nc) as tc, ExitStack() as ctx:
        const = ctx.enter_context(tc.tile_pool(name="const", bufs=1))
        state = ctx.enter_context(tc.tile_pool(name="state", bufs=2))
        work = ctx.enter_context(tc.tile_pool(name="work", bufs=2))
        plpool = ctx.enter_context(tc.tile_pool(name="plpool", bufs=1))
        psum = ctx.enter_context(tc.tile_pool(name="psum", bufs=8, space="PSUM"))
        dram = ctx.enter_context(tc.tile_pool(name="dram", bufs=3, space="DRAM"))

        pid = nc.gpsimd.partition_id()
        own_half = pid // 4          # which batch half holds this core's columns
        other_half = (pid // 4 + 1) % 2
        csrc = pid % 4 * BL          # column offset of our slice inside that half
        # store dst: the block whose half == own_half writes cols [0,BL)
        dst_off = [own_half * BL, other_half * BL]

        # ---- constants into SBUF
        whh_sb = const.tile([128, KT * 1024], bf16, name="whh_sb")
        nc.sync.dma_start(
            whh_sb.rearrange("p (k m) -> p k m", k=KT),
            whh_d.ap().rearrange("(k p) m -> p k m", p=128),
        )
        wih0_sb = const.tile([128, 1024], bf16, name="wih0_sb")
        nc.sync.dma_start(wih0_sb[:], wih0_d.ap()[:, :])
        wihm_sb = const.tile([77, 1024], bf16, name="wihm_sb")
        nc.sync.dma_start(wihm_sb[:], wihm_d.ap()[:, :])
        wfc1_sb = const.tile([128, KT * 256], bf16, name="wfc1_sb")
        nc.sync.dma_start(
            wfc1_sb.rearrange("p (k m) -> p k m", k=KT),
            wfc1_d.ap().rearrange("(k p) m -> p k m", p=128),
        )
        wfc2_sb = const.tile([128, 2 * OUT], bf16, name="wfc2_sb")
        nc.sync.dma_start(
            wfc2_sb.rearrange("p (k m) -> p k m", k=2),
            wfc2_d.ap().rearrange("(k p) m -> p k m", p=128),
        )
        bg_sb = const.tile([128, 8], f32, name="bg_sb")
        nc.sync.dma_start(bg_sb[:], bgates_d.ap()[:, :])
        bz_sb = const.tile([128, 2], f32, name="bz_sb")
        nc.sync.dma_start(bz_sb[:], bz_d.ap()[:, :])
        bo_sb = const.tile([128, 2], f32, name="bo_sb")
        nc.sync.dma_start(bo_sb[:], bo_d.ap()[:, :])

        # ---- initial state per half: hcat, c, ob0/obm (persistent out tiles)
        hcat = [None, None]
        c_prev = [[None, None], [None, None]]
        ob0 = [None, None]
        obm = [None, None]
        for X in range(2):
            hg = state.tile([128, KT * HB], bf16, tag=f"hcat{X}", name=f"hcat{X}_init")
            nc.scalar.dma_start(
                hg.rearrange("p (k n) -> p k n", k=KT),
                h0_d.ap()[:, X * HB:(X + 1) * HB].rearrange("(k p) n -> p k n", p=128),
            )
            hcat[X] = hg
            for g in range(2):
                ct = state.tile([128, HB], f32, tag=f"c{X}{g}", name=f"c{X}{g}_init")
                nc.sync.dma_start(ct[:], c0_d.ap()[g * 128:(g + 1) * 128, X * HB:(X + 1) * HB])
                c_prev[X][g] = ct
            o0 = const.tile([128, HB], bf16, name=f"ob0_{X}")
            nc.sync.dma_start(o0[:], out0_d.ap()[0:128, X * HB:(X + 1) * HB])
            ob0[X] = o0
            om = const.tile([77, HB], bf16, name=f"obm_{X}")
            nc.sync.dma_start(om[0:37, :], out0_d.ap()[128:165, X * HB:(X + 1) * HB])
            nc.sync.dma_start(om[37:77, :], onehot_d.ap()[:, X * HB:(X + 1) * HB])
            obm[X] = om

        def emit_fc_out(t, X, hc):
            """fc1 -> relu -> fc2 partial -> AllGather(partials);
            returns ag_out dram tile (8 stacked [256,HB] f32 blocks)."""
            ps_z = psum.tile([128, 512], f32, tag="ps", name=f"psz_{t}_{X}")
            for mt in range(2):
                for ki in range(KT):
                    nc.tensor.matmul(
                        ps_z[:, mt * HB:(mt + 1) * HB],
                        wfc1_sb[:, ki * 256 + mt * 128: ki * 256 + (mt + 1) * 128],
                        hc[:, ki * HB:(ki + 1) * HB],
                        start=(mt == 0 and ki == 0),
                        stop=(mt == 1 and ki == KT - 1),
                    )
            zb = work.tile([128, 512], bf16, tag=f"zb{X}", name=f"zb_{t}_{X}")
            for mt in range(2):
                nc.scalar.activation(
                    zb[:, mt * HB:(mt + 1) * HB], ps_z[:, mt * HB:(mt + 1) * HB],
                    AF.Relu, bias=bz_sb[:, mt:mt + 1],
                )
            ps_o = psum.tile([128, 512], f32, tag="ps", name=f"pso_{t}_{X}")
            for mt in range(2):
                mr = MR[mt]
                for ki in range(2):
                    nc.tensor.matmul(
                        ps_o[:mr, mt * HB:mt * HB + HB],
                        wfc2_sb[:, ki * OUT + mt * 128: ki * OUT + mt * 128 + mr],
                        zb[:, ki * HB:(ki + 1) * HB],
                        start=(mt == 0 and ki == 0),
                        stop=(mt == 1 and ki == 1),
                    )
            of = work.tile([128, 512], f32, tag=f"of{X}", name=f"of_{t}_{X}")
            for mt in range(2):
                mr = MR[mt]
                nc.scalar.activation(
                    of[:mr, mt * HB:mt * HB + HB], ps_o[:mr, mt * HB:mt * HB + HB],
                    AF.Identity, bias=bo_sb[:mr, mt:mt + 1],
                )
            # padded [256, HB] so the gathered tensor has clean 128-row APs
            ag_in = dram.tile([256, HB], f32, tag=f"agin{X}", name=f"agin_{t}_{X}")
            nc.sync.dma_start(ag_in[0:128, :], of[:, 0:HB])
            nc.sync.dma_start(ag_in[128:165, :], of[:37, HB:2 * HB])
            ag_out = dram.tile([2048, HB], f32, tag=f"agout{X}", name=f"agout_{t}_{X}",
                               addr_space="Shared")
            nc.gpsimd.collective_compute(
                "AllGather", mybir.AluOpType.bypass, replica_groups=RG,
                ins=[ag_in.opt()], outs=[ag_out.opt()],
            )
            return ag_out

        def emit_whh(t, X, hc):
            """W_hh k-chains into 4 packed psum banks: bank b = (mt 2b, 2b+1)."""
            gb = []
            for b in range(4):
                ps = psum.tile([128, 512], f32, tag="ps", name=f"psg_{t}_{X}_{b}")
                gb.append(ps)
                for sub in range(2):
                    mt = b * 2 + sub
                    for ki in range(KT):
                        nc.tensor.matmul(
                            ps[:, sub * HB:(sub + 1) * HB],
                            whh_sb[:, ki * 1024 + mt * 128: ki * 1024 + (mt + 1) * 128],
                            hc[:, ki * HB:(ki + 1) * HB],
                            start=(sub == 0 and ki == 0),
                            stop=False,
                        )
            return gb

        def emit_ag_consume(t, X, ag_out):
            """Load gathered fc2 partials, DVE-sum to out(t-1), cast to bf16."""
            pl = plpool.tile([128, 16 * HB], f32, tag=f"pl{X}", name=f"pl_{t}_{X}")
            nc.sync.dma_start(
                pl.rearrange("p (r q n) -> p r q n", r=8, q=2),
                ag_out.rearrange("(r q p) n -> p r q n", q=2, p=128),
            )
            s0 = work.tile([128, HB], f32, tag=f"s0{X}", name=f"s0_{t}_{X}")
            nc.vector.tensor_add(s0[:], pl[:, 0:HB], pl[:, 2 * HB:3 * HB])
            for r in range(2, 8):
                nc.vector.tensor_add(s0[:], s0[:], pl[:, (2 * r) * HB:(2 * r + 1) * HB])
            s1 = work.tile([37, HB], f32, tag=f"s1{X}", name=f"s1_{t}_{X}")
            nc.vector.tensor_add(s1[:], pl[0:37, HB:2 * HB], pl[0:37, 3 * HB:4 * HB])
            for r in range(2, 8):
                nc.vector.tensor_add(s1[:], s1[:],
                                     pl[0:37, (2 * r + 1) * HB:(2 * r + 2) * HB])
            nc.vector.tensor_copy(ob0[X][:], s0[:])
            nc.vector.tensor_copy(obm[X][0:37, :], s1[:])
            return s0, s1

        def emit_outs_store(t, X, s0, s1, store_t):
            # gpsimd-only (dynamic register offsets); emitted AFTER the
            # AllGather trigger so its wait doesn't delay the AG.
            nc.gpsimd.dma_start(
                outs_d.ap()[store_t, 0:128, ds(dst_off[X], BL)],
                s0[:, ds(csrc, BL)],
            )
            nc.gpsimd.dma_start(
                outs_d.ap()[store_t, 128:165, ds(dst_off[X], BL)],
                s1[:, ds(csrc, BL)],
            )

        def emit_tail(t, X, gb):
            for b in range(4):
                for sub in range(2):
                    mt = b * 2 + sub
                    dst = gb[b][:, sub * HB:(sub + 1) * HB]
                    nc.tensor.matmul(dst, wih0_sb[:, mt * 128:(mt + 1) * 128],
                                     ob0[X][:], start=False, stop=False)
            for b in range(4):
                for sub in range(2):
                    mt = b * 2 + sub
                    dst = gb[b][:, sub * HB:(sub + 1) * HB]
                    nc.tensor.matmul(dst, wihm_sb[:, mt * 128:(mt + 1) * 128],
                                     obm[X][:], start=False, stop=(sub == 1))

        def emit_lstm_ag(t, X, gb):
            """LSTM cell from gate banks -> h(t+1) slice -> AllGather; returns
            (hb_out dram tile, new c tiles)."""
            hn = work.tile([128, 512], bf16, tag=f"hn{X}", name=f"hn_{t}_{X}")
            cn_new = [None, None]
            for g in range(2):
                bi, bo_ = gb[2 * g], gb[2 * g + 1]
                si = work.tile([128, HB], f32, tag=f"si{X}", name=f"si_{t}_{X}_{g}")
                nc.scalar.activation(si[:], bi[:, 0:HB], AF.Sigmoid,
                                     bias=bg_sb[:, 4 * g + 0: 4 * g + 1])
                sf = work.tile([128, HB], f32, tag=f"sf{X}", name=f"sf_{t}_{X}_{g}")
                nc.scalar.activation(sf[:], bi[:, HB:2 * HB], AF.Sigmoid,
                                     bias=bg_sb[:, 4 * g + 1: 4 * g + 2])
                tg = work.tile([128, HB], f32, tag=f"tg{X}", name=f"tg_{t}_{X}_{g}")
                nc.scalar.activation(tg[:], bo_[:, 0:HB], AF.Tanh,
                                     bias=bg_sb[:, 4 * g + 2: 4 * g + 3])
                so = work.tile([128, HB], f32, tag=f"so{X}", name=f"so_{t}_{X}_{g}")
                nc.scalar.activation(so[:], bo_[:, HB:2 * HB], AF.Sigmoid,
                                     bias=bg_sb[:, 4 * g + 3: 4 * g + 4])
                m1 = work.tile([128, HB], f32, tag=f"m1{X}", name=f"m1_{t}_{X}_{g}")
                nc.vector.tensor_mul(m1[:], si[:], tg[:])
                m2 = work.tile([128, HB], f32, tag=f"m2{X}", name=f"m2_{t}_{X}_{g}")
                nc.vector.tensor_mul(m2[:], sf[:], c_prev[X][g][:])
                cn = state.tile([128, HB], f32, tag=f"c{X}{g}", name=f"c{X}{g}_{t}")
                nc.vector.tensor_add(cn[:], m1[:], m2[:])
                cn_new[g] = cn
                th = work.tile([128, HB], f32, tag=f"th{X}", name=f"th_{t}_{X}_{g}")
                nc.scalar.activation(th[:], cn[:], AF.Tanh)
                nc.vector.tensor_mul(hn[:, g * HB:(g + 1) * HB], so[:], th[:])
            hb_in = dram.tile([256, HB], bf16, tag=f"hbin{X}", name=f"hbin_{t}_{X}")
            nc.sync.dma_start(
                hb_in.rearrange("(g p) n -> p g n", p=128),
                hn.rearrange("p (g n) -> p g n", g=2),
            )
            hb_out = dram.tile([H, HB], bf16, tag=f"hbout{X}", name=f"hbout_{t}_{X}",
                               addr_space="Shared")
            nc.gpsimd.collective_compute(
                "AllGather", mybir.AluOpType.bypass, replica_groups=RG,
                ins=[hb_in.opt()], outs=[hb_out.opt()],
            )
            return hb_out, cn_new

        def emit_hcat_load(t, X, hb_out):
            hg = state.tile([128, KT * HB], bf16, tag=f"hcat{X}", name=f"hcat{X}_{t}")
            half_rows = (KT // 2) * 128
            for piece, eng in ((0, nc.scalar), (1, nc.scalar)):
                eng.dma_start(
                    hg[:, piece * 8 * HB:(piece + 1) * 8 * HB].rearrange(
                        "p (k n) -> p k n", k=8),
                    hb_out[piece * half_rows:(piece + 1) * half_rows, :].rearrange(
                        "(k p) n -> p k n", p=128),
                )
            return hg

        # ================= main loop =================
        # The AllGather result for half X is DMA-loaded during the OTHER
        # half's next block: by then the AG is (nearly) done, so the load's
        # semaphore wait doesn't head-of-line-block the scalar queue.
        pending_h = [None, None]
        for t in range(L):
            for X in range(2):
                hc = hcat[X]
                if t > 0:
                    ag_out = emit_fc_out(t, X, hc)
                O = 1 - X
                if pending_h[O] is not None:
                    hcat[O] = emit_hcat_load(t, O, pending_h[O])
                    pending_h[O] = None
                gb = emit_whh(t, X, hc)
                if t > 0:
                    s0, s1 = emit_ag_consume(t, X, ag_out)
                emit_tail(t, X, gb)
                hb_out, cn = emit_lstm_ag(t, X, gb)
                c_prev[X] = cn
                pending_h[X] = hb_out
                if t > 0:
                    emit_outs_store(t, X, s0, s1, t - 1)

        # ================= epilogue: out(L-1) =================
        for X in range(2):
            O = 1 - X
            if pending_h[O] is not None:
                hcat[O] = emit_hcat_load(L + X, O, pending_h[O])
                pending_h[O] = None
            ag_out = emit_fc_out(L + X, X, hcat[X])
            s0, s1 = emit_ag_consume(L + X, X, ag_out)
            emit_outs_store(L + X, X, s0, s1, L - 1)

    nc.compile()
    return nc


def _prepare_in_maps(inputs):
    bf = ml_dtypes.bfloat16
    f = {k: np.asarray(v) for k, v in inputs.items()}
    W_enc = f["W_enc"].astype(np.float32)
    b_enc = f["b_enc"].astype(np.float32)
    W_ih = f["W_ih"].astype(np.float32)
    b_ih = f["b_ih"].astype(np.float32)
    W_hh = f["W_hh"].astype(np.float32)
    b_hh = f["b_hh"].astype(np.float32)
    W_fc1 = f["W_fc1"].astype(np.float32)
    b_fc1 = f["b_fc1"].astype(np.float32)
    W_fc2 = f["W_fc2"].astype(np.float32)
    b_fc2 = f["b_fc2"].astype(np.float32)
    W_inh = f["W_inh"].astype(np.float32)
    b_inh = f["b_inh"].astype(np.float32)
    W_inc = f["W_inc"].astype(np.float32)
    b_inc = f["b_inc"].astype(np.float32)
    labels = f["labels"].astype(np.int64)
    x = f["inputs"].astype(np.float32)

    frame0 = x.reshape(B, OUT)
    h0 = frame0 @ W_inh.T + b_inh            # [B, H]
    c0 = frame0 @ W_inc.T + b_inc            # [B, H]
    onehot = np.zeros((NCLS, B), np.float32)
    onehot[labels, np.arange(B)] = 1.0
    M1 = W_ih[:, OUT:] @ W_enc               # [4H, NCLS]
    bias_gates = b_ih + b_hh + W_ih[:, OUT:] @ b_enc  # [4H]

    in_maps = []
    for j in range(NC):
        mt = np.arange(8)
        gt, g = mt % 4, mt // 4
        rows = (gt[:, None] * H + j * 256 + g[:, None] * 128
                + np.arange(128)[None, :]).reshape(-1)
        zrows = j * 256 + np.arange(256)
        bg = bias_gates[rows].reshape(8, 128).T.copy()          # [128, 8]
        bzv = b_fc1[zrows].reshape(2, 128).T.copy()             # [128, 2]
        bov = np.zeros((128, 2), np.float32)
        bov[:, 0] = b_fc2[:128] / NC
        bov[:MR[1], 1] = b_fc2[128:] / NC
        wihm = np.concatenate(
            [W_ih[rows, 128:OUT].T, M1[rows].T], axis=0)        # [77, 1024]
        in_maps.append({
            "whh": np.ascontiguousarray(W_hh[rows].T).astype(bf),
            "wih0": np.ascontiguousarray(W_ih[rows, :128].T).astype(bf),
            "wihm": np.ascontiguousarray(wihm).astype(bf),
            "wfc1": np.ascontiguousarray(W_fc1[zrows].T).astype(bf),
            "wfc2": np.ascontiguousarray(W_fc2[:, zrows].T).astype(bf),
            "onehot": onehot.astype(bf),
            "bgates": bg,
            "bz": bzv,
            "bo": bov,
            "h0": np.ascontiguousarray(h0.T).astype(bf),
            "c0": np.ascontiguousarray(c0.T[zrows]).astype(np.float32),
            "out0": np.ascontiguousarray(frame0.T).astype(bf),
        })
    return in_maps


def _get_program(L):
    if L not in _CACHE:
        _CACHE[L] = _build(L)
    return _CACHE[L]


def kernel(**inputs):
    from concourse.bass_utils import run_bass_kernel_spmd

    L = int(np.asarray(inputs["length"]))
    x = np.asarray(inputs["inputs"])
    Bq, J, D = x.shape
    assert (Bq, J * D) == (B, OUT)

    nc = _get_program(L)
    in_maps = _prepare_in_maps(inputs)
    res = run_bass_kernel_spmd(nc, in_maps, core_ids=list(range(NC)))
    # core j returns [L, OUT, 2*BL]; cols 0:BL hold batch columns j*BL:(j+1)*BL
    full = np.concatenate([res.results[j]["outs"][:, :, :BL] for j in range(NC)],
                          axis=2)
    out = np.transpose(full, (2, 0, 1)).reshape(B, L, J, D).astype(np.float32)
    return out
